# revision 27
# baseline (speedup 1.0000x reference)
"""GAT attention layer (EEGGraphAttentionLayer) for Trainium2, 8 NeuronCores.

reference math:
    Wh = h @ w                         # (8192, 64)
    e  = leaky_relu((Wh@a_src) + (Wh@a_dst).T, slope=0.2)   # (8192, 8192)
    att = where(adj > 0, e, -1e12)
    out = softmax(att, axis=1)

Sharding: rows of adj/out across 8 cores (1024 rows each); row softmax is
core-local. Each core recomputes the column-score vector s2 = h @ (w@a_dst)
(an N-vector) from the full h (4MB) instead of communicating. h and w are
passed host-transposed (pure layout change, no host arithmetic) so the
feature dim sits on SBUF partitions for the PE contractions.

Per-core device pipeline (row tile = [128, 8192], C = 2^-40 exact):
    wa12 = wT.T @ [a_src|a_dst]                      (PE)
    s1c[:, t] = C * (hsT_tile.T @ wa1)               (PE)   per-row bias
    BC2[p, j] = C * s2[j] via (C*wa2 bcast).T @ hT   (PE)   16 x 512 chunks
    e'  = Prelu(BC2 + C*s1_i, alpha=0.2)             (ACT)  == C * e
    att'= min(e', adj halves)                        (DVE)  masked -> adj<=0
    p   = Exp(2^40*att' - MSHIFT), S = rowsum        (ACT)  masked -> 0.0
    out = p * (1/S)                                  (DVE 2x tensor_scalar)

Key tricks:
  - min(C*e, adj): |C*e| <= ~2e-11 is far below any |adj| that occurs, so
    min selects C*e where adj > 0 and adj (<= 0) where masked; the masked
    entries then underflow to exactly 0.0 in the exp, matching the
    reference's -1e12 masking. C is a power of 2, so scaling is exact.
  - Fixed softmax shift MSHIFT (softmax is shift-invariant; scores are
    bounded to ~[-4, 19]) removes the row-max reduction entirely.
  - Prelu-for-1-tile-ahead emission overlaps ACT with DVE across tiles;
    adj loads ride the SP HWDGE ring, stores the ACT ring.
"""
import os
import sys

for _p in (
    "/opt/trn_rl_repo",
    "/root/.axon_site/_ro/trn_rl_repo",
):
    if os.path.isdir(_p) and _p not in sys.path:
        sys.path.append(_p)

import numpy as np


def _install_profile_shim():
    """bass_utils' trace path imports antenv.axon_hooks, which this image
    lacks. Provide it (with the ctypes hook into libaxon if available) so a
    BASS_TRACE=1 run profiles instead of crashing. No-op on any failure."""
    import contextlib
    import ctypes
    import types

    if "antenv.axon_hooks" in sys.modules:
        return
    try:
        import antenv
    except ImportError:
        return

    def _make_hook(so_path):
        try:
            lib = ctypes.CDLL(so_path)
        except OSError:
            return None
        if not hasattr(lib, "axon_start_nrt_profile"):
            return None
        lib.axon_start_nrt_profile.argtypes = [
            ctypes.POINTER(ctypes.c_int64),
            ctypes.c_size_t,
        ]
        lib.axon_start_nrt_profile.restype = ctypes.c_int64
        lib.axon_stop_nrt_profile.argtypes = [ctypes.c_char_p]
        lib.axon_stop_nrt_profile.restype = ctypes.c_int64

        @contextlib.contextmanager
        def _hook(output_dir, device_ids):
            import jax

            jax.devices()
            if device_ids:
                ids = (ctypes.c_int64 * len(device_ids))(*device_ids)
                rc = lib.axon_start_nrt_profile(ids, len(device_ids))
            else:
                rc = lib.axon_start_nrt_profile(None, 0)
            if rc != 0:
                raise RuntimeError(f"axon_start_nrt_profile rc={rc}")
            try:
                yield
            finally:
                n = lib.axon_stop_nrt_profile(str(output_dir).encode())
                print(f"profile: {n} file(s) -> {output_dir}", file=sys.stderr)

        return _hook

    hook = [_make_hook("/opt/axon/libaxon_pjrt.so")]
    mod = types.ModuleType("antenv.axon_hooks")
    mod.set_axon_ntff_profile_hook = lambda h: hook.__setitem__(0, h)
    mod.get_axon_ntff_profile_hook = lambda: hook[0]
    sys.modules["antenv.axon_hooks"] = mod
    antenv.axon_hooks = mod


try:
    _install_profile_shim()
except Exception:
    pass

import concourse.bacc as bacc
import concourse.tile as tile
import concourse.bass as bass
from concourse import mybir
from concourse.bass_utils import run_bass_kernel_spmd

N, F_IN, F_OUT = 8192, 128, 64
NCORES = 8
R = N // NCORES          # rows per core (1024)
P = 128                  # SBUF partitions
RT = R // P              # row tiles per core (8)
C = 2.0 ** -40           # exact scale-down of scores
CI = 2.0 ** 40
MSHIFT = 32.0            # fixed softmax shift: scores e are in ~[-4, 19]
ALPHA = 0.2              # leaky relu negative slope
F32 = mybir.dt.float32
AF = mybir.ActivationFunctionType
ALU = mybir.AluOpType

_CACHED_NC = None
LAST_RESULT = None       # BassKernelResults of the most recent run (for tests)


def build_nc():
    nc = bacc.Bacc("TRN2", target_bir_lowering=False)
    hT_d = nc.dram_tensor("hT", [F_IN, N], F32, kind="ExternalInput")
    hsT_d = nc.dram_tensor("hsT", [F_IN, R], F32, kind="ExternalInput")
    adj_d = nc.dram_tensor("adj", [R, N], F32, kind="ExternalInput")
    wT_d = nc.dram_tensor("wT", [F_OUT, F_IN], F32, kind="ExternalInput")
    a_d = nc.dram_tensor("a", [2 * F_OUT, 1], F32, kind="ExternalInput")
    out_d = nc.dram_tensor("out", [R, N], F32, kind="ExternalOutput")

    with tile.TileContext(nc) as tc:
        with (
            tc.tile_pool(name="persist", bufs=1) as persist,
            tc.tile_pool(name="setup", bufs=2) as setup,
            tc.tile_pool(name="hTp", bufs=2) as hTp,
            tc.tile_pool(name="psB", bufs=4, space="PSUM") as psB,
            tc.tile_pool(name="psS", bufs=1, space="PSUM") as psS,
            tc.tile_pool(name="adjp", bufs=6) as adjp,
            tc.tile_pool(name="ep", bufs=2) as ep,
            tc.tile_pool(name="small", bufs=4) as small,
        ):
            # --------- setup: s1 (per-row bias) and BC2 (C*s2 broadcast) ------
            wT_sb = persist.tile([F_OUT, F_IN], F32)
            nc.scalar.dma_start(out=wT_sb, in_=wT_d[:, :])
            # a2[o, j] = a[j*64 + o]: a_src / a_dst as two columns
            a2 = persist.tile([F_OUT, 2], F32)
            a_t = a_d.tensor if hasattr(a_d, "tensor") else a_d
            nc.scalar.dma_start(
                out=a2, in_=bass.AP(tensor=a_t, offset=0, ap=[[1, F_OUT], [F_OUT, 2]])
            )
            hsT_sb = persist.tile([P, R], F32)
            nc.scalar.dma_start(out=hsT_sb, in_=hsT_d[:, :])
            hTs = []
            hT_dmas = []
            for g in range(8):
                hTc = hTp.tile([P, N // 8], F32, tag="hTc")
                hT_dmas.append(nc.scalar.dma_start(
                    out=hTc, in_=hT_d[:, g * (N // 8):(g + 1) * (N // 8)]
                ))
                hTs.append(hTc)

            # wa12[:, j] = w @ (a_src if j==0 else a_dst), one K=64 matmul
            ps_wa = psS.tile([P, 2], F32, tag="pswa")
            nc.tensor.matmul(ps_wa, lhsT=wT_sb, rhs=a2, start=True, stop=True)
            wa12 = persist.tile([P, 2], F32)
            nc.scalar.copy(wa12, ps_wa)
            wa1 = wa12[:, 0:1]
            wa2 = wa12[:, 1:2]

            # W2B[f, p] = C * wa2[f]  (stationary matrix for the BC2 matmuls)
            ones = persist.tile([P, P], F32)
            nc.vector.memset(ones, 1.0)
            w2b = persist.tile([P, P], F32)
            nc.vector.tensor_scalar(
                out=w2b, in0=ones, scalar1=wa2, scalar2=C,
                op0=ALU.mult, op1=ALU.mult,
            )
            wa1c = persist.tile([P, 1], F32)
            nc.vector.tensor_scalar(
                out=wa1c, in0=wa1, scalar1=C, scalar2=None, op0=ALU.mult
            )

            # s1c[r, t] = C * s1[t*128 + r]  for this core's 8 row tiles
            ps_s1 = psS.tile([P, RT], F32)
            for t in range(RT):
                nc.tensor.matmul(
                    ps_s1[:, t:t + 1], lhsT=hsT_sb[:, t * P:(t + 1) * P],
                    rhs=wa1c, start=True, stop=True,
                )
            s1c = persist.tile([P, RT], F32)
            nc.scalar.copy(s1c, ps_s1)

            negm = persist.tile([P, 1], F32)
            nc.vector.memset(negm, -MSHIFT)

            # BC2[p, j] = C * s2[j] for all p  (16 chunks of 512 columns)
            bc2 = persist.tile([P, N], F32)
            for cg in range(16):
                psb = psB.tile([P, 512], F32, tag="psb")
                nc.tensor.matmul(
                    psb, lhsT=w2b,
                    rhs=hTs[cg // 2][:, (cg % 2) * 512:(cg % 2) * 512 + 512],
                    start=True, stop=True,
                )
                if cg % 2 == 0:
                    nc.vector.tensor_copy(bc2[:, cg * 512:(cg + 1) * 512], psb)
                else:
                    nc.scalar.copy(bc2[:, cg * 512:(cg + 1) * 512], psb)

            # adj loads: SP HWDGE ring, half-width tiles for deeper pipelining.
            # The first load is gated on the hT loads so the setup stream gets
            # full HBM bandwidth; the SP-ring FIFO delays the rest with it.
            H = N // 2
            adjts = []
            for t in range(RT):
                halves = []
                for hx in range(2):
                    adjh = adjp.tile([P, H], F32, tag="adjh")
                    nc.sync.dma_start(
                        out=adjh, in_=adj_d[t * P:(t + 1) * P, hx * H:(hx + 1) * H]
                    )
                    halves.append(adjh)
                adjts.append(halves)

            # ---------------- main loop over row tiles (sw-pipelined) ---------
            # chain per tile:  Prelu(ACT) -> min(DVE) -> Exp+accum(ACT)
            #                  -> recip+scale(DVE) -> store
            # Prelu for tile t+1 is emitted before tile t's min/exp so ACT and
            # DVE overlap across tiles. Softmax shift is the constant MSHIFT
            # (shift-invariant; scores are bounded), so there is no row-max
            # reduction and no cross-engine scalar dependency.
            def emit_prelu(t):
                et = ep.tile([P, N], F32, tag="et")
                for hx in range(2):
                    sl = slice(hx * H, (hx + 1) * H)
                    nc.scalar.activation(
                        out=et[:, sl], in_=bc2[:, sl], func=AF.Prelu,
                        bias=s1c[:, t:t + 1], scale=1.0, alpha=ALPHA,
                    )
                return et

            ets = {0: emit_prelu(0)}
            for t in range(RT):
                if t + 1 < RT:
                    ets[t + 1] = emit_prelu(t + 1)
                et = ets.pop(t)
                S2 = small.tile([P, 2], F32, tag="S2")
                for hx, adjh in ((0, adjts[t][0]), (1, adjts[t][1])):
                    sl = slice(hx * H, (hx + 1) * H)
                    # att' = min(e', adj) in place; adj half frees afterwards
                    nc.vector.tensor_tensor(
                        out=et[:, sl], in0=et[:, sl], in1=adjh, op=ALU.min
                    )
                    # p = exp(2^40*att' - MSHIFT) in place, S2 half = rowsum
                    nc.scalar.activation(
                        out=et[:, sl], in_=et[:, sl], func=AF.Exp,
                        bias=negm[:, 0:1], scale=CI,
                        accum_out=S2[:, hx:hx + 1],
                    )
                S = small.tile([P, 1], F32, tag="S")
                nc.vector.tensor_scalar(
                    out=S, in0=S2[:, 0:1], scalar1=S2[:, 1:2], scalar2=None,
                    op0=ALU.add,
                )
                rs = small.tile([P, 1], F32, tag="rs")
                nc.vector.reciprocal(rs, S)
                for hx in range(2):
                    sl = slice(hx * H, (hx + 1) * H)
                    nc.vector.tensor_scalar(
                        out=et[:, sl], in0=et[:, sl], scalar1=rs[:, 0:1],
                        scalar2=None, op0=ALU.mult,
                    )
                    nc.scalar.dma_start(
                        out=out_d[t * P:(t + 1) * P, sl], in_=et[:, sl]
                    )

    nc.compile()
    return nc


def kernel(h, adj, w, a):
    global _CACHED_NC, LAST_RESULT
    h = np.ascontiguousarray(h, dtype=np.float32)
    adj = np.ascontiguousarray(adj, dtype=np.float32)
    w = np.ascontiguousarray(w, dtype=np.float32)
    a = np.ascontiguousarray(a, dtype=np.float32)

    if _CACHED_NC is None:
        _CACHED_NC = build_nc()
    nc = _CACHED_NC

    hT = np.ascontiguousarray(h.T)
    wT = np.ascontiguousarray(w.T)
    in_maps = [
        {
            "hT": hT,
            "hsT": np.ascontiguousarray(hT[:, i * R:(i + 1) * R]),
            "adj": np.ascontiguousarray(adj[i * R:(i + 1) * R]),
            "wT": wT,
            "a": a,
        }
        for i in range(NCORES)
    ]
    res = run_bass_kernel_spmd(nc, in_maps, core_ids=list(range(NCORES)))
    LAST_RESULT = res
    return np.concatenate([r["out"] for r in res.results], axis=0)


# revision 28
# speedup vs baseline: 1.2154x; 1.2154x over previous
"""GAT attention layer (EEGGraphAttentionLayer) for Trainium2, 8 NeuronCores.

reference math:
    Wh = h @ w                         # (8192, 64)
    e  = leaky_relu((Wh@a_src) + (Wh@a_dst).T, slope=0.2)   # (8192, 8192)
    att = where(adj > 0, e, -1e12)
    out = softmax(att, axis=1)

Sharding: rows of adj/out across 8 cores (1024 rows each); row softmax is
core-local. Each core recomputes the column-score vector s2 = h @ (w@a_dst)
(an N-vector) from the full h (4MB) instead of communicating. h and w are
passed host-transposed (pure layout change, no host arithmetic) so the
feature dim sits on SBUF partitions for the PE contractions.

Per-core device pipeline (row tile = [128, 8192], C = 2^-40 exact):
    wa12 = wT.T @ [a_src|a_dst]                      (PE)
    s1c[:, t] = C * (hsT_tile.T @ wa1)               (PE)   per-row bias
    BC2[p, j] = C * s2[j] via (C*wa2 bcast).T @ hT   (PE)   16 x 512 chunks
    e'  = Prelu(BC2 + C*s1_i, alpha=0.2)             (ACT)  == C * e
    att'= min(e', adj halves)                        (DVE)  masked -> adj<=0
    p   = Exp(2^40*att' - MSHIFT), S = rowsum        (ACT)  masked -> 0.0
    out = p * (1/S)                                  (DVE 2x tensor_scalar)

Key tricks:
  - min(C*e, adj): |C*e| <= ~2e-11 is far below any |adj| that occurs, so
    min selects C*e where adj > 0 and adj (<= 0) where masked; the masked
    entries then underflow to exactly 0.0 in the exp, matching the
    reference's -1e12 masking. C is a power of 2, so scaling is exact.
  - Fixed softmax shift MSHIFT (softmax is shift-invariant; scores are
    bounded to ~[-4, 19]) removes the row-max reduction entirely.
  - Prelu-for-1-tile-ahead emission overlaps ACT with DVE across tiles;
    adj loads ride the SP HWDGE ring, stores the ACT ring.
"""
import os
import sys

for _p in (
    "/opt/trn_rl_repo",
    "/root/.axon_site/_ro/trn_rl_repo",
):
    if os.path.isdir(_p) and _p not in sys.path:
        sys.path.append(_p)

import numpy as np


def _install_profile_shim():
    """bass_utils' trace path imports antenv.axon_hooks, which this image
    lacks. Provide it (with the ctypes hook into libaxon if available) so a
    BASS_TRACE=1 run profiles instead of crashing. No-op on any failure."""
    import contextlib
    import ctypes
    import types

    if "antenv.axon_hooks" in sys.modules:
        return
    try:
        import antenv
    except ImportError:
        return

    def _make_hook(so_path):
        try:
            lib = ctypes.CDLL(so_path)
        except OSError:
            return None
        if not hasattr(lib, "axon_start_nrt_profile"):
            return None
        lib.axon_start_nrt_profile.argtypes = [
            ctypes.POINTER(ctypes.c_int64),
            ctypes.c_size_t,
        ]
        lib.axon_start_nrt_profile.restype = ctypes.c_int64
        lib.axon_stop_nrt_profile.argtypes = [ctypes.c_char_p]
        lib.axon_stop_nrt_profile.restype = ctypes.c_int64

        @contextlib.contextmanager
        def _hook(output_dir, device_ids):
            import jax

            jax.devices()
            if device_ids:
                ids = (ctypes.c_int64 * len(device_ids))(*device_ids)
                rc = lib.axon_start_nrt_profile(ids, len(device_ids))
            else:
                rc = lib.axon_start_nrt_profile(None, 0)
            if rc != 0:
                raise RuntimeError(f"axon_start_nrt_profile rc={rc}")
            try:
                yield
            finally:
                n = lib.axon_stop_nrt_profile(str(output_dir).encode())
                print(f"profile: {n} file(s) -> {output_dir}", file=sys.stderr)

        return _hook

    hook = [_make_hook("/opt/axon/libaxon_pjrt.so")]
    mod = types.ModuleType("antenv.axon_hooks")
    mod.set_axon_ntff_profile_hook = lambda h: hook.__setitem__(0, h)
    mod.get_axon_ntff_profile_hook = lambda: hook[0]
    sys.modules["antenv.axon_hooks"] = mod
    antenv.axon_hooks = mod


try:
    _install_profile_shim()
except Exception:
    pass

import concourse.bacc as bacc
import concourse.tile as tile
import concourse.bass as bass
from concourse import mybir
from concourse.bass_utils import run_bass_kernel_spmd

N, F_IN, F_OUT = 8192, 128, 64
NCORES = 8
R = N // NCORES          # rows per core (1024)
P = 128                  # SBUF partitions
RT = R // P              # row tiles per core (8)
C = 2.0 ** -40           # exact scale-down of scores
CI = 2.0 ** 40
MSHIFT = 32.0            # fixed softmax shift: scores e are in ~[-4, 19]
ALPHA = 0.2              # leaky relu negative slope
F32 = mybir.dt.float32
AF = mybir.ActivationFunctionType
ALU = mybir.AluOpType

_CACHED_NC = None
LAST_RESULT = None       # BassKernelResults of the most recent run (for tests)


def build_nc():
    nc = bacc.Bacc("TRN2", target_bir_lowering=False)
    hT_d = nc.dram_tensor("hT", [F_IN, N], F32, kind="ExternalInput")
    hsT_d = nc.dram_tensor("hsT", [F_IN, R], F32, kind="ExternalInput")
    adj_d = nc.dram_tensor("adj", [R, N], F32, kind="ExternalInput")
    wT_d = nc.dram_tensor("wT", [F_OUT, F_IN], F32, kind="ExternalInput")
    a_d = nc.dram_tensor("a", [2 * F_OUT, 1], F32, kind="ExternalInput")
    out_d = nc.dram_tensor("out", [R, N], F32, kind="ExternalOutput")

    with tile.TileContext(nc) as tc:
        with (
            tc.tile_pool(name="persist", bufs=1) as persist,
            tc.tile_pool(name="setup", bufs=2) as setup,
            tc.tile_pool(name="hTp", bufs=2) as hTp,
            tc.tile_pool(name="psB", bufs=4, space="PSUM") as psB,
            tc.tile_pool(name="psS", bufs=1, space="PSUM") as psS,
            tc.tile_pool(name="adjp", bufs=4) as adjp,
            tc.tile_pool(name="ep", bufs=3) as ep,
            tc.tile_pool(name="small", bufs=4) as small,
        ):
            # --------- setup: s1 (per-row bias) and BC2 (C*s2 broadcast) ------
            wT_sb = persist.tile([F_OUT, F_IN], F32)
            nc.scalar.dma_start(out=wT_sb, in_=wT_d[:, :])
            # a2[o, j] = a[j*64 + o]: a_src / a_dst as two columns
            a2 = persist.tile([F_OUT, 2], F32)
            a_t = a_d.tensor if hasattr(a_d, "tensor") else a_d
            nc.scalar.dma_start(
                out=a2, in_=bass.AP(tensor=a_t, offset=0, ap=[[1, F_OUT], [F_OUT, 2]])
            )
            hsT_sb = persist.tile([P, R], F32)
            nc.scalar.dma_start(out=hsT_sb, in_=hsT_d[:, :])
            hTs = []
            hT_dmas = []
            for g in range(8):
                hTc = hTp.tile([P, N // 8], F32, tag="hTc")
                hT_dmas.append(nc.scalar.dma_start(
                    out=hTc, in_=hT_d[:, g * (N // 8):(g + 1) * (N // 8)]
                ))
                hTs.append(hTc)

            # wa12[:, j] = w @ (a_src if j==0 else a_dst), one K=64 matmul
            ps_wa = psS.tile([P, 2], F32, tag="pswa")
            nc.tensor.matmul(ps_wa, lhsT=wT_sb, rhs=a2, start=True, stop=True)
            wa12 = persist.tile([P, 2], F32)
            nc.scalar.copy(wa12, ps_wa)
            wa1 = wa12[:, 0:1]
            wa2 = wa12[:, 1:2]

            # W2B[f, p] = C * wa2[f]  (stationary matrix for the BC2 matmuls)
            ones = persist.tile([P, P], F32)
            nc.vector.memset(ones, 1.0)
            w2b = persist.tile([P, P], F32)
            nc.vector.tensor_scalar(
                out=w2b, in0=ones, scalar1=wa2, scalar2=C,
                op0=ALU.mult, op1=ALU.mult,
            )
            wa1c = persist.tile([P, 1], F32)
            nc.vector.tensor_scalar(
                out=wa1c, in0=wa1, scalar1=C, scalar2=None, op0=ALU.mult
            )

            # s1c[r, t] = C * s1[t*128 + r]  for this core's 8 row tiles
            ps_s1 = psS.tile([P, RT], F32)
            for t in range(RT):
                nc.tensor.matmul(
                    ps_s1[:, t:t + 1], lhsT=hsT_sb[:, t * P:(t + 1) * P],
                    rhs=wa1c, start=True, stop=True,
                )
            s1c = persist.tile([P, RT], F32)
            nc.scalar.copy(s1c, ps_s1)

            negm = persist.tile([P, 1], F32)
            nc.vector.memset(negm, -MSHIFT)

            # BC2[p, j] = C * s2[j] for all p  (16 chunks of 512 columns)
            bc2 = persist.tile([P, N], F32)
            for cg in range(16):
                psb = psB.tile([P, 512], F32, tag="psb")
                nc.tensor.matmul(
                    psb, lhsT=w2b,
                    rhs=hTs[cg // 2][:, (cg % 2) * 512:(cg % 2) * 512 + 512],
                    start=True, stop=True,
                )
                if cg % 2 == 0:
                    nc.vector.tensor_copy(bc2[:, cg * 512:(cg + 1) * 512], psb)
                else:
                    nc.scalar.copy(bc2[:, cg * 512:(cg + 1) * 512], psb)

            # adj loads: SP HWDGE ring, half-width tiles for deeper pipelining.
            # The first load is gated on the hT loads so the setup stream gets
            # full HBM bandwidth; the SP-ring FIFO delays the rest with it.
            H = N // 2
            adjts = []
            for t in range(RT):
                halves = []
                for hx in range(2):
                    adjh = adjp.tile([P, H], F32, tag="adjh")
                    nc.sync.dma_start(
                        out=adjh, in_=adj_d[t * P:(t + 1) * P, hx * H:(hx + 1) * H]
                    )
                    halves.append(adjh)
                adjts.append(halves)

            # ---------------- main loop over row tiles (sw-pipelined) ---------
            # chain per tile:  Prelu(ACT) -> min(DVE) -> Exp+accum(ACT)
            #                  -> recip+scale(DVE) -> store
            # Prelu for tile t+1 is emitted before tile t's min/exp so ACT and
            # DVE overlap across tiles. Softmax shift is the constant MSHIFT
            # (shift-invariant; scores are bounded), so there is no row-max
            # reduction and no cross-engine scalar dependency.
            def emit_prelu(t):
                et = ep.tile([P, N], F32, tag="et")
                for hx in range(2):
                    sl = slice(hx * H, (hx + 1) * H)
                    nc.scalar.activation(
                        out=et[:, sl], in_=bc2[:, sl], func=AF.Prelu,
                        bias=s1c[:, t:t + 1], scale=1.0, alpha=ALPHA,
                    )
                return et

            ets = {0: emit_prelu(0)}
            for t in range(RT):
                if t + 1 < RT:
                    ets[t + 1] = emit_prelu(t + 1)
                et = ets.pop(t)
                S2 = small.tile([P, 2], F32, tag="S2")
                for hx, adjh in ((0, adjts[t][0]), (1, adjts[t][1])):
                    sl = slice(hx * H, (hx + 1) * H)
                    # att' = min(e', adj) in place; adj half frees afterwards
                    nc.vector.tensor_tensor(
                        out=et[:, sl], in0=et[:, sl], in1=adjh, op=ALU.min
                    )
                    # p = exp(2^40*att' - MSHIFT) in place, S2 half = rowsum
                    nc.scalar.activation(
                        out=et[:, sl], in_=et[:, sl], func=AF.Exp,
                        bias=negm[:, 0:1], scale=CI,
                        accum_out=S2[:, hx:hx + 1],
                    )
                S = small.tile([P, 1], F32, tag="S")
                nc.vector.tensor_scalar(
                    out=S, in0=S2[:, 0:1], scalar1=S2[:, 1:2], scalar2=None,
                    op0=ALU.add,
                )
                rs = small.tile([P, 1], F32, tag="rs")
                nc.vector.reciprocal(rs, S)
                for hx in range(2):
                    sl = slice(hx * H, (hx + 1) * H)
                    nc.vector.tensor_scalar(
                        out=et[:, sl], in0=et[:, sl], scalar1=rs[:, 0:1],
                        scalar2=None, op0=ALU.mult,
                    )
                    nc.scalar.dma_start(
                        out=out_d[t * P:(t + 1) * P, sl], in_=et[:, sl]
                    )

    nc.compile()
    return nc


def kernel(h, adj, w, a):
    global _CACHED_NC, LAST_RESULT
    h = np.ascontiguousarray(h, dtype=np.float32)
    adj = np.ascontiguousarray(adj, dtype=np.float32)
    w = np.ascontiguousarray(w, dtype=np.float32)
    a = np.ascontiguousarray(a, dtype=np.float32)

    if _CACHED_NC is None:
        _CACHED_NC = build_nc()
    nc = _CACHED_NC

    hT = np.ascontiguousarray(h.T)
    wT = np.ascontiguousarray(w.T)
    in_maps = [
        {
            "hT": hT,
            "hsT": np.ascontiguousarray(hT[:, i * R:(i + 1) * R]),
            "adj": np.ascontiguousarray(adj[i * R:(i + 1) * R]),
            "wT": wT,
            "a": a,
        }
        for i in range(NCORES)
    ]
    res = run_bass_kernel_spmd(nc, in_maps, core_ids=list(range(NCORES)))
    LAST_RESULT = res
    return np.concatenate([r["out"] for r in res.results], axis=0)


# revision 29
# speedup vs baseline: 1.2987x; 1.0685x over previous
"""GAT attention layer (EEGGraphAttentionLayer) for Trainium2, 8 NeuronCores.

reference math:
    Wh = h @ w                         # (8192, 64)
    e  = leaky_relu((Wh@a_src) + (Wh@a_dst).T, slope=0.2)   # (8192, 8192)
    att = where(adj > 0, e, -1e12)
    out = softmax(att, axis=1)

Sharding: rows of adj/out across 8 cores (1024 rows each); row softmax is
core-local. Each core recomputes the column-score vector s2 = h @ (w@a_dst)
(an N-vector) from the full h (4MB) instead of communicating. h and w are
passed host-transposed (pure layout change, no host arithmetic) so the
feature dim sits on SBUF partitions for the PE contractions.

Per-core device pipeline (row tile = [128, 8192], C = 2^-40 exact):
    wa12 = wT.T @ [a_src|a_dst]                      (PE)
    s1c[:, t] = C * (hsT_tile.T @ wa1)               (PE)   per-row bias
    BC2[p, j] = C * s2[j] via (C*wa2 bcast).T @ hT   (PE)   16 x 512 chunks
    e'  = Prelu(BC2 + C*s1_i, alpha=0.2)             (ACT)  == C * e
    att'= min(e', adj halves)                        (DVE)  masked -> adj<=0
    p   = Exp(2^40*att' - MSHIFT), S = rowsum        (ACT)  masked -> 0.0
    out = p * (1/S)                                  (DVE 2x tensor_scalar)

Key tricks:
  - min(C*e, adj): |C*e| <= ~2e-11 is far below any |adj| that occurs, so
    min selects C*e where adj > 0 and adj (<= 0) where masked; the masked
    entries then underflow to exactly 0.0 in the exp, matching the
    reference's -1e12 masking. C is a power of 2, so scaling is exact.
  - Fixed softmax shift MSHIFT (softmax is shift-invariant; scores are
    bounded to ~[-4, 19]) removes the row-max reduction entirely.
  - Prelu-for-1-tile-ahead emission overlaps ACT with DVE across tiles;
    adj loads ride the SP HWDGE ring, stores the ACT ring.
"""
import os
import sys

for _p in (
    "/opt/trn_rl_repo",
    "/root/.axon_site/_ro/trn_rl_repo",
):
    if os.path.isdir(_p) and _p not in sys.path:
        sys.path.append(_p)

import numpy as np
import ml_dtypes


def _install_profile_shim():
    """bass_utils' trace path imports antenv.axon_hooks, which this image
    lacks. Provide it (with the ctypes hook into libaxon if available) so a
    BASS_TRACE=1 run profiles instead of crashing. No-op on any failure."""
    import contextlib
    import ctypes
    import types

    if "antenv.axon_hooks" in sys.modules:
        return
    try:
        import antenv
    except ImportError:
        return

    def _make_hook(so_path):
        try:
            lib = ctypes.CDLL(so_path)
        except OSError:
            return None
        if not hasattr(lib, "axon_start_nrt_profile"):
            return None
        lib.axon_start_nrt_profile.argtypes = [
            ctypes.POINTER(ctypes.c_int64),
            ctypes.c_size_t,
        ]
        lib.axon_start_nrt_profile.restype = ctypes.c_int64
        lib.axon_stop_nrt_profile.argtypes = [ctypes.c_char_p]
        lib.axon_stop_nrt_profile.restype = ctypes.c_int64

        @contextlib.contextmanager
        def _hook(output_dir, device_ids):
            import jax

            jax.devices()
            if device_ids:
                ids = (ctypes.c_int64 * len(device_ids))(*device_ids)
                rc = lib.axon_start_nrt_profile(ids, len(device_ids))
            else:
                rc = lib.axon_start_nrt_profile(None, 0)
            if rc != 0:
                raise RuntimeError(f"axon_start_nrt_profile rc={rc}")
            try:
                yield
            finally:
                n = lib.axon_stop_nrt_profile(str(output_dir).encode())
                print(f"profile: {n} file(s) -> {output_dir}", file=sys.stderr)

        return _hook

    hook = [_make_hook("/opt/axon/libaxon_pjrt.so")]
    mod = types.ModuleType("antenv.axon_hooks")
    mod.set_axon_ntff_profile_hook = lambda h: hook.__setitem__(0, h)
    mod.get_axon_ntff_profile_hook = lambda: hook[0]
    sys.modules["antenv.axon_hooks"] = mod
    antenv.axon_hooks = mod


try:
    _install_profile_shim()
except Exception:
    pass

import concourse.bacc as bacc
import concourse.tile as tile
import concourse.bass as bass
from concourse import mybir
from concourse.bass_utils import run_bass_kernel_spmd

N, F_IN, F_OUT = 8192, 128, 64
NCORES = 8
R = N // NCORES          # rows per core (1024)
P = 128                  # SBUF partitions
RT = R // P              # row tiles per core (8)
C = 2.0 ** -40           # exact scale-down of scores
CI = 2.0 ** 40
MSHIFT = 32.0            # fixed softmax shift: scores e are in ~[-4, 19]
ALPHA = 0.2              # leaky relu negative slope
F32 = mybir.dt.float32
AF = mybir.ActivationFunctionType
ALU = mybir.AluOpType

_CACHED_NC = None
LAST_RESULT = None       # BassKernelResults of the most recent run (for tests)


def build_nc():
    nc = bacc.Bacc("TRN2", target_bir_lowering=False)
    hT_d = nc.dram_tensor("hT", [F_IN, N], F32, kind="ExternalInput")
    hsT_d = nc.dram_tensor("hsT", [F_IN, R], F32, kind="ExternalInput")
    adj_d = nc.dram_tensor("adj", [R, N], mybir.dt.bfloat16, kind="ExternalInput")
    wT_d = nc.dram_tensor("wT", [F_OUT, F_IN], F32, kind="ExternalInput")
    a_d = nc.dram_tensor("a", [2 * F_OUT, 1], F32, kind="ExternalInput")
    out_d = nc.dram_tensor("out", [R, N], F32, kind="ExternalOutput")

    with tile.TileContext(nc) as tc:
        with (
            tc.tile_pool(name="persist", bufs=1) as persist,
            tc.tile_pool(name="setup", bufs=2) as setup,
            tc.tile_pool(name="hTp", bufs=2) as hTp,
            tc.tile_pool(name="psB", bufs=4, space="PSUM") as psB,
            tc.tile_pool(name="psS", bufs=1, space="PSUM") as psS,
            tc.tile_pool(name="adjp", bufs=6) as adjp,
            tc.tile_pool(name="ep", bufs=3) as ep,
            tc.tile_pool(name="small", bufs=4) as small,
        ):
            # --------- setup: s1 (per-row bias) and BC2 (C*s2 broadcast) ------
            wT_sb = persist.tile([F_OUT, F_IN], F32)
            nc.scalar.dma_start(out=wT_sb, in_=wT_d[:, :])
            # a2[o, j] = a[j*64 + o]: a_src / a_dst as two columns
            a2 = persist.tile([F_OUT, 2], F32)
            a_t = a_d.tensor if hasattr(a_d, "tensor") else a_d
            nc.scalar.dma_start(
                out=a2, in_=bass.AP(tensor=a_t, offset=0, ap=[[1, F_OUT], [F_OUT, 2]])
            )
            hsT_sb = persist.tile([P, R], F32)
            nc.scalar.dma_start(out=hsT_sb, in_=hsT_d[:, :])
            hTs = []
            hT_dmas = []
            for g in range(8):
                hTc = hTp.tile([P, N // 8], F32, tag="hTc")
                hT_dmas.append(nc.scalar.dma_start(
                    out=hTc, in_=hT_d[:, g * (N // 8):(g + 1) * (N // 8)]
                ))
                hTs.append(hTc)

            # wa12[:, j] = w @ (a_src if j==0 else a_dst), one K=64 matmul
            ps_wa = psS.tile([P, 2], F32, tag="pswa")
            nc.tensor.matmul(ps_wa, lhsT=wT_sb, rhs=a2, start=True, stop=True)
            wa12 = persist.tile([P, 2], F32)
            nc.scalar.copy(wa12, ps_wa)
            wa1 = wa12[:, 0:1]
            wa2 = wa12[:, 1:2]

            # W2B[f, p] = C * wa2[f]  (stationary matrix for the BC2 matmuls)
            ones = persist.tile([P, P], F32)
            nc.vector.memset(ones, 1.0)
            w2b = persist.tile([P, P], F32)
            nc.vector.tensor_scalar(
                out=w2b, in0=ones, scalar1=wa2, scalar2=C,
                op0=ALU.mult, op1=ALU.mult,
            )
            wa1c = persist.tile([P, 1], F32)
            nc.vector.tensor_scalar(
                out=wa1c, in0=wa1, scalar1=C, scalar2=None, op0=ALU.mult
            )

            # s1c[r, t] = C * s1[t*128 + r]  for this core's 8 row tiles
            ps_s1 = psS.tile([P, RT], F32)
            for t in range(RT):
                nc.tensor.matmul(
                    ps_s1[:, t:t + 1], lhsT=hsT_sb[:, t * P:(t + 1) * P],
                    rhs=wa1c, start=True, stop=True,
                )
            s1c = persist.tile([P, RT], F32)
            nc.scalar.copy(s1c, ps_s1)

            negm = persist.tile([P, 1], F32)
            nc.vector.memset(negm, -MSHIFT)

            # BC2[p, j] = C * s2[j] for all p  (16 chunks of 512 columns)
            bc2 = persist.tile([P, N], F32)
            for cg in range(16):
                psb = psB.tile([P, 512], F32, tag="psb")
                nc.tensor.matmul(
                    psb, lhsT=w2b,
                    rhs=hTs[cg // 2][:, (cg % 2) * 512:(cg % 2) * 512 + 512],
                    start=True, stop=True,
                )
                if cg % 2 == 0:
                    nc.vector.tensor_copy(bc2[:, cg * 512:(cg + 1) * 512], psb)
                else:
                    nc.scalar.copy(bc2[:, cg * 512:(cg + 1) * 512], psb)

            # adj loads: SP HWDGE ring, half-width tiles for deeper pipelining.
            # The first load is gated on the hT loads so the setup stream gets
            # full HBM bandwidth; the SP-ring FIFO delays the rest with it.
            H = N // 2
            adjts = []
            for t in range(RT):
                halves = []
                for hx in range(2):
                    adjh = adjp.tile([P, H], mybir.dt.bfloat16, tag="adjh")
                    nc.sync.dma_start(
                        out=adjh, in_=adj_d[t * P:(t + 1) * P, hx * H:(hx + 1) * H]
                    )
                    halves.append(adjh)
                adjts.append(halves)

            # ---------------- main loop over row tiles (sw-pipelined) ---------
            # chain per tile:  Prelu(ACT) -> min(DVE) -> Exp+accum(ACT)
            #                  -> recip+scale(DVE) -> store
            # Prelu for tile t+1 is emitted before tile t's min/exp so ACT and
            # DVE overlap across tiles. Softmax shift is the constant MSHIFT
            # (shift-invariant; scores are bounded), so there is no row-max
            # reduction and no cross-engine scalar dependency.
            def emit_prelu(t):
                et = ep.tile([P, N], F32, tag="et")
                for hx in range(2):
                    sl = slice(hx * H, (hx + 1) * H)
                    nc.scalar.activation(
                        out=et[:, sl], in_=bc2[:, sl], func=AF.Prelu,
                        bias=s1c[:, t:t + 1], scale=1.0, alpha=ALPHA,
                    )
                return et

            ets = {0: emit_prelu(0)}
            for t in range(RT):
                if t + 1 < RT:
                    ets[t + 1] = emit_prelu(t + 1)
                et = ets.pop(t)
                S2 = small.tile([P, 2], F32, tag="S2")
                for hx, adjh in ((0, adjts[t][0]), (1, adjts[t][1])):
                    sl = slice(hx * H, (hx + 1) * H)
                    # att' = min(e', adj) in place; adj half frees afterwards
                    nc.vector.tensor_tensor(
                        out=et[:, sl], in0=et[:, sl], in1=adjh, op=ALU.min
                    )
                    # p = exp(2^40*att' - MSHIFT) in place, S2 half = rowsum
                    nc.scalar.activation(
                        out=et[:, sl], in_=et[:, sl], func=AF.Exp,
                        bias=negm[:, 0:1], scale=CI,
                        accum_out=S2[:, hx:hx + 1],
                    )
                S = small.tile([P, 1], F32, tag="S")
                nc.vector.tensor_scalar(
                    out=S, in0=S2[:, 0:1], scalar1=S2[:, 1:2], scalar2=None,
                    op0=ALU.add,
                )
                rs = small.tile([P, 1], F32, tag="rs")
                nc.vector.reciprocal(rs, S)
                for hx in range(2):
                    sl = slice(hx * H, (hx + 1) * H)
                    nc.vector.tensor_scalar(
                        out=et[:, sl], in0=et[:, sl], scalar1=rs[:, 0:1],
                        scalar2=None, op0=ALU.mult,
                    )
                    nc.scalar.dma_start(
                        out=out_d[t * P:(t + 1) * P, sl], in_=et[:, sl]
                    )

    nc.compile()
    return nc


def kernel(h, adj, w, a):
    global _CACHED_NC, LAST_RESULT
    h = np.ascontiguousarray(h, dtype=np.float32)
    adj = np.ascontiguousarray(adj, dtype=np.float32)
    w = np.ascontiguousarray(w, dtype=np.float32)
    a = np.ascontiguousarray(a, dtype=np.float32)

    if _CACHED_NC is None:
        _CACHED_NC = build_nc()
    nc = _CACHED_NC

    hT = np.ascontiguousarray(h.T)
    wT = np.ascontiguousarray(w.T)
    in_maps = [
        {
            "hT": hT,
            "hsT": np.ascontiguousarray(hT[:, i * R:(i + 1) * R]),
            "adj": np.ascontiguousarray(
                adj[i * R:(i + 1) * R].astype(ml_dtypes.bfloat16)
            ),
            "wT": wT,
            "a": a,
        }
        for i in range(NCORES)
    ]
    res = run_bass_kernel_spmd(nc, in_maps, core_ids=list(range(NCORES)))
    LAST_RESULT = res
    return np.concatenate([r["out"] for r in res.results], axis=0)
